# revision 2
# baseline (speedup 1.0000x reference)
"""Trainium2 Bass kernel for CrossInnerProductWithBuyer — final (int8 stream + ACT-convert split).

Per batch b (B=16384, E=128): out[b] = concat(win@c, -(neg@c), buy@c).

Host quantizes a = concat(win,-neg,buy) [B,75,E] to int8 with one scale
per batch, folded into fp16 ct = fp16(c * s).  Rel err ~8.9e-3 (gate
2e-2).

Streams (SBUF-ingest fabric ~410 GB/s on written bytes is the cap):
  - rows 0..43  (aq): SWDGE cast-DMA int8->fp16, multiplied by the DVE
  - rows 44..74 (az): raw int8 (1B/elem write), Scalar engine converts
    int8->fp16, DVE multiplies.  (A GPSIMD tensor_mul offload was tried
    and reverted: GPSIMD shares its SBUF port with the DVE, which
    slowed every DVE op ~25% — a net loss.)
Engine busy budget per core: stream ~79us, DVE ~85us (critical),
Scalar ~66us, PE ~71us.

The z-pair load for tiles 0/1 leads the SWDGE FIFO (so the first
convert isn't late), then tile 0's first cast-DMA half (so the DVE
starts ~13us).  Stores ride the idle sync queue in three pieces.
"""

import sys

if "/opt/trn_rl_repo" not in sys.path:
    sys.path.insert(0, "/opt/trn_rl_repo")

from contextlib import ExitStack

import numpy as np

import concourse.bass as bass
import concourse.mybir as mybir
import concourse.tile as tile
from concourse import bacc, bass_utils

B, W, N, E = 16384, 10, 64, 128
NCORES = 8
BS = B // NCORES            # 2048 batches per core
PT = 128                    # batches per tile
NT = BS // PT               # 16 tiles per core
R = W + N + 1               # 75 output rows per batch
F = R * PT                  # 9600 product columns per tile
CHUNK = 480                 # matmul N; 20 * 480 == F
NCH = F // CHUNK            # 20 chunks -> PSUM partitions 0..19
KF = 44                     # rows streamed as cast-DMA fp16
KZ = R - KF                 # rows streamed int8 + ACT-converted (31)
FZ = KZ * PT
FQ = KF * PT

FP32 = mybir.dt.float32
FP16 = mybir.dt.float16
INT8 = mybir.dt.int8


def _build(bs: int = BS) -> bass.Bass:
    nt = bs // PT
    nc = bacc.Bacc("TRN2", target_bir_lowering=False, debug=False,
                   num_devices=NCORES)
    aq = nc.dram_tensor("aq", [E, bs * KF], INT8, kind="ExternalInput").ap()
    az = nc.dram_tensor("az", [E, bs * KZ], INT8, kind="ExternalInput").ap()
    ct = nc.dram_tensor("ct", [E, bs], FP16, kind="ExternalInput").ap()
    out = nc.dram_tensor("out", [NCH, nt * CHUNK], FP16,
                         kind="ExternalOutput").ap()

    with tile.TileContext(nc) as tc, ExitStack() as ctx:
        apool = ctx.enter_context(tc.tile_pool(name="a", bufs=7))
        zpool = ctx.enter_context(tc.tile_pool(name="z", bufs=3))
        cpool = ctx.enter_context(tc.tile_pool(name="c", bufs=1))
        idpool = ctx.enter_context(tc.tile_pool(name="id", bufs=1))
        opool = ctx.enter_context(tc.tile_pool(name="o", bufs=1))
        pspool = ctx.enter_context(tc.tile_pool(name="ps", bufs=6,
                                                space="PSUM"))

        cfull = cpool.tile([E, bs], FP16)
        nc.sync.dma_start(cfull[:, 0:PT], ct[:, 0:PT])
        nc.sync.dma_start(cfull[:, PT:], ct[:, PT:])

        idt = idpool.tile([E, NCH * NCH], FP16)
        nc.vector.memset(idt[:], 0.0)
        idv = idt[:].rearrange("e (j m) -> e j m", m=NCH)
        for j in range(NCH):
            nc.vector.memset(idv[:, j, j:j + 1], 1.0)

        oacc = opool.tile([NCH, nt * CHUNK], FP16)

        atiles: dict = {}
        zbufs: dict = {}

        def ensure_a(t):
            if t not in atiles:
                atiles[t] = apool.tile([E, F], FP16, name="a")
            return atiles[t]

        def ensure_z(t):
            k = t // 2
            if k not in zbufs:
                zbufs[k] = zpool.tile([E, 2 * FZ], INT8, name="z")
                nc.gpsimd.dma_start(zbufs[k][:],
                                    az[:, 2 * k * FZ:(2 * k + 2) * FZ])
            return zbufs[k]

        def emit_convert(t):
            z = ensure_z(t)
            a = ensure_a(t)
            nc.scalar.copy(a[:, FQ:F], z[:, (t % 2) * FZ:(t % 2 + 1) * FZ])

        # FIFO head: z-pair(0,1) first (feeds the first converts), then
        # tile 0's first cast-DMA half (feeds the first DVE multiply).
        ensure_z(0)
        a0 = ensure_a(0)
        nc.gpsimd.dma_start(a0[:, 0:(KF // 2) * PT],
                            aq[:, 0:(KF // 2) * PT])
        emit_convert(0)
        emit_convert(1)

        def emit_pe(t):
            a = atiles[t]
            ps = pspool.tile([NCH, CHUNK], FP32, name="ps")
            for j in range(NCH):
                nc.tensor.matmul(ps[:], idv[:, j, :],
                                 a[:, j * CHUNK:(j + 1) * CHUNK],
                                 start=(j == 0), stop=(j == NCH - 1))
            nc.scalar.copy(oacc[:, t * CHUNK:(t + 1) * CHUNK], ps[:])
            if t == 11:
                nc.sync.dma_start(out[:, 0:12 * CHUNK],
                                  oacc[:, 0:12 * CHUNK])
            if t == 14:
                nc.sync.dma_start(out[:, 12 * CHUNK:15 * CHUNK],
                                  oacc[:, 12 * CHUNK:15 * CHUNK])

        # The PE/copy stage for tile t-1 is emitted during iteration t
        # (one behind) so converts/multiplies lead in program order.
        aq_splits = {0: 2, 1: 2, nt - 1: 3}
        for t in range(nt):
            a = ensure_a(t)
            av = a[:].rearrange("e (r b) -> e r b", b=PT)
            cb = cfull[:, t * PT:(t + 1) * PT].unsqueeze(1)

            splits = aq_splits.get(t, 1)
            rq = KF // splits
            for q in range(splits):
                r0, r1 = q * rq, ((q + 1) * rq if q < splits - 1 else KF)
                if not (t == 0 and q == 0):     # tile 0 half 0 pre-issued
                    nc.gpsimd.dma_start(
                        a[:, r0 * PT:r1 * PT],
                        aq[:, t * FQ + r0 * PT:t * FQ + r1 * PT])
                nc.vector.tensor_mul(
                    av[:, r0:r1, :], av[:, r0:r1, :],
                    cb.broadcast_to([E, R, PT])[:, r0:r1, :])
            nc.vector.tensor_mul(
                av[:, KF:R, :], av[:, KF:R, :],
                cb.broadcast_to([E, R, PT])[:, KF:R, :])

            if t + 2 < nt:
                emit_convert(t + 2)
            if t >= 1:
                emit_pe(t - 1)
        emit_pe(nt - 1)
        nc.sync.dma_start(out[:, 15 * CHUNK:], oacc[:, 15 * CHUNK:])
    nc.compile()
    return nc


_NC_CACHE: dict = {}


def _get_nc(bs: int = BS) -> bass.Bass:
    if bs not in _NC_CACHE:
        _NC_CACHE[bs] = _build(bs)
    return _NC_CACHE[bs]


def _prep_core(center, windows, negs, buy):
    """Per-batch int8 quantization of a; scale folded into fp16 c."""
    bs = center.shape[0]
    a = np.concatenate([
        windows.reshape(bs, W, E),
        -negs.reshape(bs, N, E),
        buy.reshape(bs, 1, E),
    ], axis=1).astype(np.float32)                # [bs, 75, E]
    s = np.abs(a).reshape(bs, -1).max(axis=1) / 127.0    # [bs]
    s[s == 0] = 1.0
    q = np.clip(np.rint(a / s[:, None, None]), -127, 127).astype(np.int8)
    qt = q.reshape(bs // PT, PT, R, E).transpose(3, 0, 2, 1)  # [E,nt,R,PT]
    aqt = np.ascontiguousarray(qt[:, :, :KF, :].reshape(E, bs * KF))
    azt = np.ascontiguousarray(qt[:, :, KF:, :].reshape(E, bs * KZ))
    cpre = (center.reshape(bs, E) * s[:, None]).astype(np.float16)
    ctt = np.ascontiguousarray(cpre.T)
    return aqt, azt, ctt


def _shard_inputs(center_vec, windows_vecs, neg_vecs, buy_vec):
    center_vec = np.asarray(center_vec, dtype=np.float32)
    windows_vecs = np.asarray(windows_vecs, dtype=np.float32)
    neg_vecs = np.asarray(neg_vecs, dtype=np.float32)
    buy_vec = np.asarray(buy_vec, dtype=np.float32)
    in_maps = []
    for i in range(NCORES):
        sl = slice(i * BS, (i + 1) * BS)
        aqt, azt, ctt = _prep_core(center_vec[sl], windows_vecs[sl],
                                   neg_vecs[sl], buy_vec[sl])
        in_maps.append({"aq": aqt, "az": azt, "ct": ctt})
    return in_maps


def run(center_vec, windows_vecs, neg_vecs, buy_vec, trace: bool = False):
    """Run on 8 NeuronCores; returns (full_output, BassKernelResults)."""
    nc = _get_nc()
    in_maps = _shard_inputs(center_vec, windows_vecs, neg_vecs, buy_vec)
    res = bass_utils.run_bass_kernel_spmd(
        nc, in_maps, list(range(NCORES)), trace=trace)
    parts = []
    for i in range(NCORES):
        o = res.results[i]["out"].reshape(NCH, NT, CHUNK)
        o = o.transpose(1, 0, 2).reshape(NT, R, PT)   # [nt, r, b]
        parts.append(np.ascontiguousarray(
            o.transpose(0, 2, 1)).reshape(BS, R).astype(np.float32))
    full = np.concatenate(parts, axis=0)
    return full.reshape(B, R, 1), res


def kernel(center_vec, windows_vecs, neg_vecs, buy_vec):
    out, _ = run(center_vec, windows_vecs, neg_vecs, buy_vec)
    return out


# revision 3
# speedup vs baseline: 1.0072x; 1.0072x over previous
"""Trainium2 Bass kernel for CrossInnerProductWithBuyer — final (int8 stream + ACT-convert split).

Per batch b (B=16384, E=128): out[b] = concat(win@c, -(neg@c), buy@c).

Host quantizes a = concat(win,-neg,buy) [B,75,E] to int8 with one scale
per batch, folded into fp16 ct = fp16(c * s).  Rel err ~8.9e-3 (gate
2e-2).

Streams (SBUF-ingest fabric ~410 GB/s on written bytes is the cap):
  - rows 0..43  (aq): SWDGE cast-DMA int8->fp16, multiplied by the DVE
  - rows 44..74 (az): raw int8 (1B/elem write), Scalar engine converts
    int8->fp16, DVE multiplies.  (A GPSIMD tensor_mul offload was tried
    and reverted: GPSIMD shares its SBUF port with the DVE, which
    slowed every DVE op ~25% — a net loss.)
Engine busy budget per core: stream ~79us, DVE ~85us (critical),
Scalar ~66us, PE ~71us.

The z-pair load for tiles 0/1 leads the SWDGE FIFO (so the first
convert isn't late), then tile 0's first cast-DMA half (so the DVE
starts ~13us).  Stores ride the idle sync queue in three pieces.
"""

import sys

if "/opt/trn_rl_repo" not in sys.path:
    sys.path.insert(0, "/opt/trn_rl_repo")

from contextlib import ExitStack

import numpy as np

import concourse.bass as bass
import concourse.mybir as mybir
import concourse.tile as tile
from concourse import bacc, bass_utils

B, W, N, E = 16384, 10, 64, 128
NCORES = 8
BS = B // NCORES            # 2048 batches per core
PT = 128                    # batches per tile
NT = BS // PT               # 16 tiles per core
R = W + N + 1               # 75 output rows per batch
F = R * PT                  # 9600 product columns per tile
CHUNK = 480                 # matmul N; 20 * 480 == F
NCH = F // CHUNK            # 20 chunks -> PSUM partitions 0..19
KF = 44                     # rows streamed as cast-DMA fp16
KZ = R - KF                 # rows streamed int8 + ACT-converted (31)
FZ = KZ * PT
FQ = KF * PT

FP32 = mybir.dt.float32
FP16 = mybir.dt.float16
INT8 = mybir.dt.int8


def _build(bs: int = BS) -> bass.Bass:
    nt = bs // PT
    nc = bacc.Bacc("TRN2", target_bir_lowering=False, debug=False,
                   num_devices=NCORES)
    aq = nc.dram_tensor("aq", [E, bs * KF], INT8, kind="ExternalInput").ap()
    az = nc.dram_tensor("az", [E, bs * KZ], INT8, kind="ExternalInput").ap()
    ct = nc.dram_tensor("ct", [E, bs], FP16, kind="ExternalInput").ap()
    out = nc.dram_tensor("out", [NCH, nt * CHUNK], FP16,
                         kind="ExternalOutput").ap()

    with tile.TileContext(nc) as tc, ExitStack() as ctx:
        apool = ctx.enter_context(tc.tile_pool(name="a", bufs=8))
        zpool = ctx.enter_context(tc.tile_pool(name="z", bufs=3))
        cpool = ctx.enter_context(tc.tile_pool(name="c", bufs=1))
        idpool = ctx.enter_context(tc.tile_pool(name="id", bufs=1))
        opool = ctx.enter_context(tc.tile_pool(name="o", bufs=1))
        pspool = ctx.enter_context(tc.tile_pool(name="ps", bufs=6,
                                                space="PSUM"))

        cfull = cpool.tile([E, bs], FP16)
        nc.sync.dma_start(cfull[:, 0:PT], ct[:, 0:PT])
        nc.sync.dma_start(cfull[:, PT:], ct[:, PT:])

        idt = idpool.tile([E, NCH * NCH], FP16)
        nc.vector.memset(idt[:], 0.0)
        idv = idt[:].rearrange("e (j m) -> e j m", m=NCH)
        for j in range(NCH):
            nc.vector.memset(idv[:, j, j:j + 1], 1.0)

        oacc = opool.tile([NCH, nt * CHUNK], FP16)

        atiles: dict = {}
        zbufs: dict = {}

        def ensure_a(t):
            if t not in atiles:
                atiles[t] = apool.tile([E, F], FP16, name="a")
            return atiles[t]

        def ensure_z(t):
            k = t // 2
            if k not in zbufs:
                zbufs[k] = zpool.tile([E, 2 * FZ], INT8, name="z")
                nc.gpsimd.dma_start(zbufs[k][:],
                                    az[:, 2 * k * FZ:(2 * k + 2) * FZ])
            return zbufs[k]

        def emit_convert(t):
            z = ensure_z(t)
            a = ensure_a(t)
            nc.scalar.copy(a[:, FQ:F], z[:, (t % 2) * FZ:(t % 2 + 1) * FZ])

        # FIFO head: z-pair(0,1) first (feeds the first converts), then
        # tile 0's first cast-DMA half (feeds the first DVE multiply).
        ensure_z(0)
        a0 = ensure_a(0)
        nc.gpsimd.dma_start(a0[:, 0:(KF // 2) * PT],
                            aq[:, 0:(KF // 2) * PT])
        emit_convert(0)
        emit_convert(1)

        def emit_pe(t):
            a = atiles[t]
            ps = pspool.tile([NCH, CHUNK], FP32, name="ps")
            for j in range(NCH):
                nc.tensor.matmul(ps[:], idv[:, j, :],
                                 a[:, j * CHUNK:(j + 1) * CHUNK],
                                 start=(j == 0), stop=(j == NCH - 1))
            nc.scalar.copy(oacc[:, t * CHUNK:(t + 1) * CHUNK], ps[:])
            if t == 11:
                nc.sync.dma_start(out[:, 0:12 * CHUNK],
                                  oacc[:, 0:12 * CHUNK])
            if t == 14:
                nc.sync.dma_start(out[:, 12 * CHUNK:15 * CHUNK],
                                  oacc[:, 12 * CHUNK:15 * CHUNK])

        # The PE/copy stage for tile t-1 is emitted during iteration t
        # (one behind) so converts/multiplies lead in program order.
        aq_splits = {0: 2, 1: 2, nt - 1: 3}
        for t in range(nt):
            a = ensure_a(t)
            av = a[:].rearrange("e (r b) -> e r b", b=PT)
            cb = cfull[:, t * PT:(t + 1) * PT].unsqueeze(1)

            splits = aq_splits.get(t, 1)
            rq = KF // splits
            for q in range(splits):
                r0, r1 = q * rq, ((q + 1) * rq if q < splits - 1 else KF)
                if not (t == 0 and q == 0):     # tile 0 half 0 pre-issued
                    nc.gpsimd.dma_start(
                        a[:, r0 * PT:r1 * PT],
                        aq[:, t * FQ + r0 * PT:t * FQ + r1 * PT])
                nc.vector.tensor_mul(
                    av[:, r0:r1, :], av[:, r0:r1, :],
                    cb.broadcast_to([E, R, PT])[:, r0:r1, :])
            nc.vector.tensor_mul(
                av[:, KF:R, :], av[:, KF:R, :],
                cb.broadcast_to([E, R, PT])[:, KF:R, :])

            if t + 2 < nt:
                emit_convert(t + 2)
            if t >= 1:
                emit_pe(t - 1)
        emit_pe(nt - 1)
        nc.sync.dma_start(out[:, 15 * CHUNK:], oacc[:, 15 * CHUNK:])
    nc.compile()
    return nc


_NC_CACHE: dict = {}


def _get_nc(bs: int = BS) -> bass.Bass:
    if bs not in _NC_CACHE:
        _NC_CACHE[bs] = _build(bs)
    return _NC_CACHE[bs]


def _prep_core(center, windows, negs, buy):
    """Per-batch int8 quantization of a; scale folded into fp16 c."""
    bs = center.shape[0]
    a = np.concatenate([
        windows.reshape(bs, W, E),
        -negs.reshape(bs, N, E),
        buy.reshape(bs, 1, E),
    ], axis=1).astype(np.float32)                # [bs, 75, E]
    s = np.abs(a).reshape(bs, -1).max(axis=1) / 127.0    # [bs]
    s[s == 0] = 1.0
    q = np.clip(np.rint(a / s[:, None, None]), -127, 127).astype(np.int8)
    qt = q.reshape(bs // PT, PT, R, E).transpose(3, 0, 2, 1)  # [E,nt,R,PT]
    aqt = np.ascontiguousarray(qt[:, :, :KF, :].reshape(E, bs * KF))
    azt = np.ascontiguousarray(qt[:, :, KF:, :].reshape(E, bs * KZ))
    cpre = (center.reshape(bs, E) * s[:, None]).astype(np.float16)
    ctt = np.ascontiguousarray(cpre.T)
    return aqt, azt, ctt


def _shard_inputs(center_vec, windows_vecs, neg_vecs, buy_vec):
    center_vec = np.asarray(center_vec, dtype=np.float32)
    windows_vecs = np.asarray(windows_vecs, dtype=np.float32)
    neg_vecs = np.asarray(neg_vecs, dtype=np.float32)
    buy_vec = np.asarray(buy_vec, dtype=np.float32)
    in_maps = []
    for i in range(NCORES):
        sl = slice(i * BS, (i + 1) * BS)
        aqt, azt, ctt = _prep_core(center_vec[sl], windows_vecs[sl],
                                   neg_vecs[sl], buy_vec[sl])
        in_maps.append({"aq": aqt, "az": azt, "ct": ctt})
    return in_maps


def run(center_vec, windows_vecs, neg_vecs, buy_vec, trace: bool = False):
    """Run on 8 NeuronCores; returns (full_output, BassKernelResults)."""
    nc = _get_nc()
    in_maps = _shard_inputs(center_vec, windows_vecs, neg_vecs, buy_vec)
    res = bass_utils.run_bass_kernel_spmd(
        nc, in_maps, list(range(NCORES)), trace=trace)
    parts = []
    for i in range(NCORES):
        o = res.results[i]["out"].reshape(NCH, NT, CHUNK)
        o = o.transpose(1, 0, 2).reshape(NT, R, PT)   # [nt, r, b]
        parts.append(np.ascontiguousarray(
            o.transpose(0, 2, 1)).reshape(BS, R).astype(np.float32))
    full = np.concatenate(parts, axis=0)
    return full.reshape(B, R, 1), res


def kernel(center_vec, windows_vecs, neg_vecs, buy_vec):
    out, _ = run(center_vec, windows_vecs, neg_vecs, buy_vec)
    return out


# revision 4
# speedup vs baseline: 1.0122x; 1.0049x over previous
"""Trainium2 Bass kernel for CrossInnerProductWithBuyer — final (int8 stream + ACT-convert split + merged muls).

Per batch b (B=16384, E=128): out[b] = concat(win@c, -(neg@c), buy@c).

Host quantizes a = concat(win,-neg,buy) [B,75,E] to int8 with one scale
per batch, folded into fp16 ct = fp16(c * s).  Rel err ~8.9e-3 (gate
2e-2).

Streams (SBUF-ingest fabric ~410 GB/s on written bytes is the cap):
  - rows 0..43  (aq): SWDGE cast-DMA int8->fp16, multiplied by the DVE
  - rows 44..74 (az): raw int8 (1B/elem write), Scalar engine converts
    int8->fp16, DVE multiplies.  (A GPSIMD tensor_mul offload was tried
    and reverted: GPSIMD shares its SBUF port with the DVE, which
    slowed every DVE op ~25% — a net loss.)
Engine busy budget per core: stream ~79us, DVE ~85us (critical),
Scalar ~66us, PE ~71us.

The z-pair load for tiles 0/1 leads the SWDGE FIFO (so the first
convert isn't late), then tile 0's first cast-DMA half (so the DVE
starts ~13us).  Stores ride the idle sync queue in three pieces.
"""

import sys

if "/opt/trn_rl_repo" not in sys.path:
    sys.path.insert(0, "/opt/trn_rl_repo")

from contextlib import ExitStack

import numpy as np

import concourse.bass as bass
import concourse.mybir as mybir
import concourse.tile as tile
from concourse import bacc, bass_utils

B, W, N, E = 16384, 10, 64, 128
NCORES = 8
BS = B // NCORES            # 2048 batches per core
PT = 128                    # batches per tile
NT = BS // PT               # 16 tiles per core
R = W + N + 1               # 75 output rows per batch
F = R * PT                  # 9600 product columns per tile
CHUNK = 480                 # matmul N; 20 * 480 == F
NCH = F // CHUNK            # 20 chunks -> PSUM partitions 0..19
KF = 44                     # rows streamed as cast-DMA fp16
KZ = R - KF                 # rows streamed int8 + ACT-converted (31)
FZ = KZ * PT
FQ = KF * PT

FP32 = mybir.dt.float32
FP16 = mybir.dt.float16
INT8 = mybir.dt.int8


def _build(bs: int = BS) -> bass.Bass:
    nt = bs // PT
    nc = bacc.Bacc("TRN2", target_bir_lowering=False, debug=False,
                   num_devices=NCORES)
    aq = nc.dram_tensor("aq", [E, bs * KF], INT8, kind="ExternalInput").ap()
    az = nc.dram_tensor("az", [E, bs * KZ], INT8, kind="ExternalInput").ap()
    ct = nc.dram_tensor("ct", [E, bs], FP16, kind="ExternalInput").ap()
    out = nc.dram_tensor("out", [NCH, nt * CHUNK], FP16,
                         kind="ExternalOutput").ap()

    with tile.TileContext(nc) as tc, ExitStack() as ctx:
        apool = ctx.enter_context(tc.tile_pool(name="a", bufs=8))
        zpool = ctx.enter_context(tc.tile_pool(name="z", bufs=3))
        cpool = ctx.enter_context(tc.tile_pool(name="c", bufs=1))
        idpool = ctx.enter_context(tc.tile_pool(name="id", bufs=1))
        opool = ctx.enter_context(tc.tile_pool(name="o", bufs=1))
        pspool = ctx.enter_context(tc.tile_pool(name="ps", bufs=6,
                                                space="PSUM"))

        cfull = cpool.tile([E, bs], FP16)
        nc.sync.dma_start(cfull[:, 0:PT], ct[:, 0:PT])
        nc.sync.dma_start(cfull[:, PT:], ct[:, PT:])

        idt = idpool.tile([E, NCH * NCH], FP16)
        nc.vector.memset(idt[:], 0.0)
        idv = idt[:].rearrange("e (j m) -> e j m", m=NCH)
        for j in range(NCH):
            nc.vector.memset(idv[:, j, j:j + 1], 1.0)

        oacc = opool.tile([NCH, nt * CHUNK], FP16)

        atiles: dict = {}
        zbufs: dict = {}

        def ensure_a(t):
            if t not in atiles:
                atiles[t] = apool.tile([E, F], FP16, name="a")
            return atiles[t]

        def ensure_z(t):
            k = t // 2
            if k not in zbufs:
                zbufs[k] = zpool.tile([E, 2 * FZ], INT8, name="z")
                nc.gpsimd.dma_start(zbufs[k][:],
                                    az[:, 2 * k * FZ:(2 * k + 2) * FZ])
            return zbufs[k]

        def emit_convert(t):
            z = ensure_z(t)
            a = ensure_a(t)
            nc.scalar.copy(a[:, FQ:F], z[:, (t % 2) * FZ:(t % 2 + 1) * FZ])

        # FIFO head: z-pair(0,1) first (feeds the first converts), then
        # tile 0's first cast-DMA half (feeds the first DVE multiply).
        ensure_z(0)
        a0 = ensure_a(0)
        nc.gpsimd.dma_start(a0[:, 0:(KF // 2) * PT],
                            aq[:, 0:(KF // 2) * PT])
        emit_convert(0)
        emit_convert(1)

        def emit_pe(t):
            a = atiles[t]
            ps = pspool.tile([NCH, CHUNK], FP32, name="ps")
            for j in range(NCH):
                nc.tensor.matmul(ps[:], idv[:, j, :],
                                 a[:, j * CHUNK:(j + 1) * CHUNK],
                                 start=(j == 0), stop=(j == NCH - 1))
            nc.scalar.copy(oacc[:, t * CHUNK:(t + 1) * CHUNK], ps[:])
            if t == 11:
                nc.sync.dma_start(out[:, 0:12 * CHUNK],
                                  oacc[:, 0:12 * CHUNK])
            if t == 14:
                nc.sync.dma_start(out[:, 12 * CHUNK:15 * CHUNK],
                                  oacc[:, 12 * CHUNK:15 * CHUNK])

        # The PE/copy stage for tile t-1 is emitted during iteration t
        # (one behind) so converts/multiplies lead in program order.
        aq_splits = {0: 2, 1: 2, nt - 1: 3}
        for t in range(nt):
            a = ensure_a(t)
            av = a[:].rearrange("e (r b) -> e r b", b=PT)
            cb = cfull[:, t * PT:(t + 1) * PT].unsqueeze(1)

            splits = aq_splits.get(t, 1)
            rq = KF // splits
            for q in range(splits):
                r0, r1 = q * rq, ((q + 1) * rq if q < splits - 1 else KF)
                if not (t == 0 and q == 0):     # tile 0 half 0 pre-issued
                    nc.gpsimd.dma_start(
                        a[:, r0 * PT:r1 * PT],
                        aq[:, t * FQ + r0 * PT:t * FQ + r1 * PT])
                if splits > 1:
                    nc.vector.tensor_mul(
                        av[:, r0:r1, :], av[:, r0:r1, :],
                        cb.broadcast_to([E, R, PT])[:, r0:r1, :])
            if splits > 1:      # boundary tiles: az rows separately
                nc.vector.tensor_mul(
                    av[:, KF:R, :], av[:, KF:R, :],
                    cb.broadcast_to([E, R, PT])[:, KF:R, :])
            else:               # interior tiles: ONE 75-row multiply
                nc.vector.tensor_mul(
                    av[:, 0:R, :], av[:, 0:R, :],
                    cb.broadcast_to([E, R, PT])[:, 0:R, :])

            if t + 2 < nt:
                emit_convert(t + 2)
            if t >= 1:
                emit_pe(t - 1)
        emit_pe(nt - 1)
        nc.sync.dma_start(out[:, 15 * CHUNK:], oacc[:, 15 * CHUNK:])
    nc.compile()
    return nc


_NC_CACHE: dict = {}


def _get_nc(bs: int = BS) -> bass.Bass:
    if bs not in _NC_CACHE:
        _NC_CACHE[bs] = _build(bs)
    return _NC_CACHE[bs]


def _prep_core(center, windows, negs, buy):
    """Per-batch int8 quantization of a; scale folded into fp16 c."""
    bs = center.shape[0]
    a = np.concatenate([
        windows.reshape(bs, W, E),
        -negs.reshape(bs, N, E),
        buy.reshape(bs, 1, E),
    ], axis=1).astype(np.float32)                # [bs, 75, E]
    s = np.abs(a).reshape(bs, -1).max(axis=1) / 127.0    # [bs]
    s[s == 0] = 1.0
    q = np.clip(np.rint(a / s[:, None, None]), -127, 127).astype(np.int8)
    qt = q.reshape(bs // PT, PT, R, E).transpose(3, 0, 2, 1)  # [E,nt,R,PT]
    aqt = np.ascontiguousarray(qt[:, :, :KF, :].reshape(E, bs * KF))
    azt = np.ascontiguousarray(qt[:, :, KF:, :].reshape(E, bs * KZ))
    cpre = (center.reshape(bs, E) * s[:, None]).astype(np.float16)
    ctt = np.ascontiguousarray(cpre.T)
    return aqt, azt, ctt


def _shard_inputs(center_vec, windows_vecs, neg_vecs, buy_vec):
    center_vec = np.asarray(center_vec, dtype=np.float32)
    windows_vecs = np.asarray(windows_vecs, dtype=np.float32)
    neg_vecs = np.asarray(neg_vecs, dtype=np.float32)
    buy_vec = np.asarray(buy_vec, dtype=np.float32)
    in_maps = []
    for i in range(NCORES):
        sl = slice(i * BS, (i + 1) * BS)
        aqt, azt, ctt = _prep_core(center_vec[sl], windows_vecs[sl],
                                   neg_vecs[sl], buy_vec[sl])
        in_maps.append({"aq": aqt, "az": azt, "ct": ctt})
    return in_maps


def run(center_vec, windows_vecs, neg_vecs, buy_vec, trace: bool = False):
    """Run on 8 NeuronCores; returns (full_output, BassKernelResults)."""
    nc = _get_nc()
    in_maps = _shard_inputs(center_vec, windows_vecs, neg_vecs, buy_vec)
    res = bass_utils.run_bass_kernel_spmd(
        nc, in_maps, list(range(NCORES)), trace=trace)
    parts = []
    for i in range(NCORES):
        o = res.results[i]["out"].reshape(NCH, NT, CHUNK)
        o = o.transpose(1, 0, 2).reshape(NT, R, PT)   # [nt, r, b]
        parts.append(np.ascontiguousarray(
            o.transpose(0, 2, 1)).reshape(BS, R).astype(np.float32))
    full = np.concatenate(parts, axis=0)
    return full.reshape(B, R, 1), res


def kernel(center_vec, windows_vecs, neg_vecs, buy_vec):
    out, _ = run(center_vec, windows_vecs, neg_vecs, buy_vec)
    return out
